# revision 15
# baseline (speedup 1.0000x reference)
"""CrossModalMatchingNetwork Trainium2 kernel.

Full-input contract: kernel(**inputs) takes the unsharded numpy inputs and
returns the full [B, S, S] cosine-similarity output (float32).

Strategy: data-parallel over batch across 8 NeuronCores (2 batches/core).
Host-side prep transposes the big activations to [D, S] layout so the
contraction dim lands on SBUF partitions, casts them to bf16 (fp32 PSUM
accumulation), and replicates the small projection weights (pre-transposed
to [D, H]) to every core.

Per core, per batch (engine-balanced schedule):
  tT[h,s]  = sum_d WtT[d,h] * txtT[d,s] + bt[h]     (k-outer: 4 interleaved
  vT[h,s]  = sum_d WvT[d,h] * visT[d,s] + bv[h]      PSUM chains trickle-feed
                                                      from DMA at startup)
  tn2[s]   = sum_h tT[h,s]^2   (DVE squares+adds -> ones-col matmul row)
  tT      *= 1/tn   (DVE approx-recip row -> sqrt -> ones-row repl matmul;
                     DVE multiplies tT by the replicated PSUM directly)
  vn2      = per-column sums via tiny stationary=vss moving=ones matmuls,
             giving vn2 directly in COLUMN layout [s-block(P), i]
  dots     = vT.T @ tT~           (raw vT stationary; 1/tn already folded)
  out      = dots * (1/vn)[partition] applied during the PSUM->SBUF copy,
             alternating Scalar/DVE so neither queue backs up
Output is written bf16 (halves output DMA); host upcasts to float32.

DMA design (v2): host pre-packs activations into contiguous [128, k*S]
layouts so every transfer is a plain 2D DMA with 2KB+ rows.  The two
tensors gating the very first matmul (wt chunk 0, txt b0 chunk 0) ride
first on SEPARATE queues (scalar / sync) so their transfers overlap; the
rest of txt b0 is striped per-chunk across sync/gpsimd/vector in compute
need-order, vis b0 rides split in halves on sync/gpsimd, and the batch-1
bulk (tx1, vi1) is consolidated into single large DMAs.  This cuts the
DMA count ~2x (less queue issue time, fewer completion events to retire
in the fixed end-of-program semaphore-reset stream) and roughly doubles
early input bandwidth.  A 36-matmul PE warm-up bridges the first-DMA
latency with zero idle so the PE p-state is fully ramped (2.4 GHz) when
real work starts (the old schedule had a 2.5us gap that reset the clock
to 1.2 GHz for the first ~3us of real matmuls).
"""

import numpy as np
from contextlib import ExitStack

import concourse.bass as bass
import concourse.mybir as mybir
import concourse.tile as tile
from concourse import bacc
from concourse.bass import ds, ts

B, S, VD, TD, H = 16, 1024, 1024, 768, 512
NCORES = 8
BPC = B // NCORES  # batches per core
P = 128
FD = 512  # matmul moving-operand free dim (one PSUM bank of fp32)

F32 = mybir.dt.float32
BF16 = mybir.dt.bfloat16

AF = mybir.ActivationFunctionType

N_WARMUP = 36


def build(bpc=BPC, s=S, vd=VD, td=TD, h=H):
    kv, kt, mh = vd // P, td // P, h // P
    ns, ms = s // FD, s // P  # 2 free-dim halves, 8 s-blocks of 128
    CT = BF16

    nc = bacc.Bacc("TRN2", target_bir_lowering=False)
    # txt b0: chunk 0 alone (gates first matmul), chunks 1..5 singles
    tx00 = nc.dram_tensor("tx00", [P, s], CT, kind="ExternalInput")
    tx0r = nc.dram_tensor("tx0r", [kt - 1, P, s], CT, kind="ExternalInput")
    tx1 = nc.dram_tensor("tx1", [P, kt * s], CT, kind="ExternalInput")
    vi0q = nc.dram_tensor(  # vis b0 in quarters [q, P, 2*s]
        "vi0q", [4, P, (kv // 4) * s], CT, kind="ExternalInput"
    )
    vi1 = nc.dram_tensor("vi1", [P, kv * s], CT, kind="ExternalInput")
    wt0 = nc.dram_tensor("wt0", [P, h], CT, kind="ExternalInput")
    wtr = nc.dram_tensor("wtr", [P, (kt - 1) * h], CT, kind="ExternalInput")
    wva = nc.dram_tensor("wva", [P, (kv // 2) * h], CT, kind="ExternalInput")
    wvb = nc.dram_tensor("wvb", [P, (kv // 2) * h], CT, kind="ExternalInput")
    btp = nc.dram_tensor("btp", [P, mh], F32, kind="ExternalInput")
    bvp = nc.dram_tensor("bvp", [P, mh], F32, kind="ExternalInput")
    onesd = nc.dram_tensor("ones", [P, P], CT, kind="ExternalInput")
    out = nc.dram_tensor("out", [bpc, s, s], CT, kind="ExternalOutput")

    with (
        tile.TileContext(nc) as tc,
        ExitStack() as ctx,
        nc.allow_low_precision(reason="compute dtype is bf16 by design"),
    ):
        consts = ctx.enter_context(tc.tile_pool(name="consts", bufs=1))
        txt_pool = ctx.enter_context(tc.tile_pool(name="txt", bufs=1))
        vis_pool = ctx.enter_context(tc.tile_pool(name="vis", bufs=1))
        tt_pool = ctx.enter_context(tc.tile_pool(name="tt", bufs=1))
        vt_pool = ctx.enter_context(tc.tile_pool(name="vt", bufs=1))
        sq_pool = ctx.enter_context(tc.tile_pool(name="sq", bufs=1))
        row_pool = ctx.enter_context(tc.tile_pool(name="rows", bufs=2))
        col_pool = ctx.enter_context(tc.tile_pool(name="cols", bufs=1))
        out_pool = ctx.enter_context(tc.tile_pool(name="outs", bufs=3))
        ps_mm = ctx.enter_context(tc.tile_pool(name="ps_mm", bufs=6, space="PSUM"))
        ps_repl = ctx.enter_context(tc.tile_pool(name="ps_repl", bufs=2, space="PSUM"))

        # --- first-matmul gates on separate queues, issued first
        wt0_sb = consts.tile([P, h], CT, name="wt0")
        nc.scalar.dma_start(wt0_sb[:], wt0[:, :])
        tx00_sb = txt_pool.tile([P, s], CT, name="tx00")
        nc.sync.dma_start(tx00_sb[:], tx00[:, :])

        # --- remaining wt chunks per-chunk on scalar (each gates its own
        # k-group; a consolidated transfer would stall proj-t at k=1)
        wtr_sb = [
            consts.tile([P, h], CT, name=f"wt{k + 1}") for k in range(kt - 1)
        ]
        for k in range(kt - 1):
            nc.scalar.dma_start(wtr_sb[k][:], wtr[:, ds(k * h, h)])
        # wv in chunk-pairs on scalar (pair granularity keeps supply ~2.5us
        # ahead of proj-v's per-chunk demand)
        wv_sb = [
            consts.tile([P, 2 * h], CT, name=f"wvp{p}") for p in range(kv // 2)
        ]
        for p in range(kv // 4):
            nc.scalar.dma_start(wv_sb[p][:], wva[:, ds(p * 2 * h, 2 * h)])
        for p in range(kv // 4):
            nc.scalar.dma_start(
                wv_sb[kv // 4 + p][:], wvb[:, ds(p * 2 * h, 2 * h)]
            )

        # --- small constants early on scalar (tiny; needed by warm-up acts)
        bt_sb = consts.tile([P, mh], F32)
        nc.scalar.dma_start(bt_sb[:], btp[:, :])
        bv_sb = consts.tile([P, mh], F32)
        nc.scalar.dma_start(bv_sb[:], bvp[:, :])
        ones_sb = consts.tile([P, P], CT)
        nc.scalar.dma_start(ones_sb[:], onesd[:, :])
        ones_col = ones_sb[:, 0:1]
        ones_row = ones_sb[0:1, :]

        # --- txt b0 chunks 1..5 as separate tiles (per-chunk gating),
        # striped gpsimd/sync so two queues feed full-clock PE consumption
        tx0r_sb = [
            txt_pool.tile([P, s], CT, name=f"tx0_{k + 1}") for k in range(kt - 1)
        ]
        txq = [nc.gpsimd, nc.sync, nc.gpsimd, nc.sync, nc.gpsimd]
        for k in range(kt - 1):
            txq[k].dma_start(tx0r_sb[k][:], tx0r[k, :, :])

        # --- vis b0 quarters alternating sync/gpsimd
        vi0q_sb = [
            vis_pool.tile([P, (kv // 4) * s], CT, name=f"vi0q{q}")
            for q in range(4)
        ]
        viq = [nc.sync, nc.gpsimd, nc.sync, nc.gpsimd]
        for q in range(4):
            viq[q].dma_start(vi0q_sb[q][:], vi0q[q, :, :])

        # --- batch-1 bulk, consolidated single DMAs (consumed ~50us+):
        # tx1 rides scalar (after the weights), vi1 rides sync
        tx1_sb = txt_pool.tile([P, kt * s], CT, name="tx1")
        nc.scalar.dma_start(tx1_sb[:], tx1[:, :])
        vi1_sb = vis_pool.tile([P, kv * s], CT, name="vi1")
        nc.sync.dma_start(vi1_sb[:], vi1[:, :])

        # chunk accessors returning final APs (no AP re-slicing)
        def tx_ap(b, k, lo, sz):
            if b == 0:
                t = tx00_sb if k == 0 else tx0r_sb[k - 1]
                return t[:, ds(lo, sz)]
            return tx1_sb[:, ds(k * s + lo, sz)]

        def vi_ap(b, k, lo, sz):
            if b == 0:
                q, kk = divmod(k, kv // 4)
                return vi0q_sb[q][:, ds(kk * s + lo, sz)]
            return vi1_sb[:, ds(k * s + lo, sz)]

        def wt_ap(k, m):
            if k == 0:
                return wt0_sb[:, ts(m, P)]
            return wtr_sb[k - 1][:, ts(m, P)]

        def wv_ap(k, m):
            pair, kk = divmod(k, 2)
            return wv_sb[pair][:, ds(kk * h + m * P, P)]

        # PE warm-up while the first input DMAs are in flight: bridges the
        # full first-DMA latency with zero PE idle so the p-state is ramped
        # (>=3us continuous) when real work arrives.  Plus activation-table
        # prewarm covering every variant used later (Identity+bias, Copy
        # +scale, Sqrt, plain Copy) so no ACT_TABLE_LOAD stalls mid-kernel.
        warm_sb = consts.tile([P, P], CT)
        nc.vector.memset(warm_sb[:], 0.0)
        warm_f = consts.tile([1, 8], F32)
        nc.vector.memset(warm_f[:], 1.0)
        warm_c = consts.tile([P, 1], F32)
        nc.vector.memset(warm_c[:], 1.0)
        warm_ps = ps_repl.tile([P, FD], F32, tag="ps_repl")
        for _ in range(N_WARMUP):
            nc.tensor.matmul(warm_ps[:, 0:P], warm_sb[:], warm_sb[:])
        warm_o = consts.tile([P, 8], F32)
        nc.scalar.activation(warm_o[0:1, :], warm_f[:], AF.Identity)
        nc.scalar.activation(warm_o[0:1, :], warm_f[:], AF.Sqrt)
        nc.scalar.activation(warm_o[0:1, :], warm_f[:], AF.Copy)
        nc.scalar.activation(
            warm_o[:, 0:1], warm_c[:, 0:1], AF.Identity, bias=warm_c[:, 0:1]
        )
        nc.scalar.activation(
            warm_o[:, 0:1], warm_c[:, 0:1], AF.Copy, scale=warm_c[:, 0:1]
        )

        def proj_chains(kk, w_ap_f, x_ap_f, chains, pvs=None, extra=None,
                        pool=None, tag="ps_mm"):
            """k-outer interleaved PSUM accumulation chains over (n2, m) pairs.
            The PE queue is strictly in-order, so phase A runs 6 chains during
            the DMA trickle (max runnable work per arriving chunk) and phase B
            finishes the last 2 (from the ps_repl banks, which are free by
            then - avoids WAR-waiting on phase A's bias ACTs).
            `extra[k]` emits PE ops inside the stream."""
            if pvs is None:
                pvs = {}
            if pool is None:
                pool = ps_mm
            for c in chains:
                pvs[c] = pool.tile(
                    [P, FD], F32, tag=tag, name=f"pj{c[0]}_{c[1]}"
                )
            for k in range(kk):
                for n2, m in chains:
                    nc.tensor.matmul(
                        pvs[(n2, m)][:],
                        w_ap_f(k, m),
                        x_ap_f(k, n2 * FD, FD),
                        start=(k == 0),
                        stop=(k == kk - 1),
                    )
                if extra is not None and k in extra:
                    extra[k]()
            return pvs

        def proj_act(pvs, b_sb, y_sb, n2):
            sl = ds(n2 * FD, FD)
            for m in range(mh):
                nc.scalar.activation(
                    y_sb[:, m, sl], pvs[m][:], AF.Identity, bias=b_sb[:, ds(m, 1)]
                )

        def squares(y_sb, ysq_sb, n2):
            sl = ds(n2 * FD, FD)
            nc.vector.tensor_mul(
                ysq_sb[:, :, sl], y_sb[:, :, sl], y_sb[:, :, sl]
            )

        def chunk_sum(ysq_sb, yss_sb, n2):
            sl = ds(n2 * FD, FD)
            nc.vector.tensor_add(yss_sb[:, sl], ysq_sb[:, 0, sl], ysq_sb[:, 1, sl])
            for m in range(2, mh):
                nc.vector.tensor_add(yss_sb[:, sl], yss_sb[:, sl], ysq_sb[:, m, sl])

        for b in range(bpc):
            tt_sb = tt_pool.tile([P, mh, s], CT)
            vt_sb = vt_pool.tile([P, mh, s], CT)
            tsq_sb = sq_pool.tile([P, mh, s], CT, tag="tsq")
            vsq_sb = sq_pool.tile([P, mh, s], CT, tag="vsq")
            tss_sb = sq_pool.tile([P, s], CT, tag="tss")
            vss_sb = sq_pool.tile([P, s], CT, tag="vss")
            rvn_cols = col_pool.tile([P, ms], F32, tag="rvn")

            wtf = wt_ap
            txf = lambda k, lo, sz, _b=b: tx_ap(_b, k, lo, sz)  # noqa: E731
            wvf = wv_ap
            vif = lambda k, lo, sz, _b=b: vi_ap(_b, k, lo, sz)  # noqa: E731

            # --- proj-t: phase A (6 chains) then phase B (2 chains)
            chA = [(0, m) for m in range(mh)] + [(1, 0), (1, 1)]
            pts = proj_chains(kt, wtf, txf, chA)
            proj_act([pts[(0, m)] for m in range(mh)], bt_sb, tt_sb, 0)
            squares(tt_sb, tsq_sb, 0)
            proj_chains(kt, wtf, txf, [(1, 2), (1, 3)], pvs=pts,
                        pool=ps_repl, tag="ps_repl")
            proj_act([pts[(1, m)] for m in range(mh)], bt_sb, tt_sb, 1)
            squares(tt_sb, tsq_sb, 1)
            for n2 in range(ns):
                chunk_sum(tsq_sb, tss_sb, n2)

            # --- proj-v phase A, with the whole t-norm chain emitted via
            # k-hooks so it completes mid-phase: tn2 rows (k=4) -> approx
            # 1/tn2 + sqrt (k=5) -> replicate + PSUM->SBUF bf16 copy (k=6)
            rrows, srows, pn_t = [], [], []
            rtn_bc = col_pool.tile([P, s], CT, tag="rtn")

            def emit_pn():
                for n2 in range(ns):
                    sl = ds(n2 * FD, FD)
                    pn = ps_repl.tile(
                        [1, FD], F32, tag="ps_repl", name=f"pn{n2}"
                    )
                    nc.tensor.matmul(pn[:], ones_col, tss_sb[:, sl])
                    pn_t.append(pn)

            def emit_rsqrt():
                for n2 in range(ns):
                    rrow = row_pool.tile([1, FD], F32, tag=f"rr{n2}")
                    nc.vector.reciprocal_approx_fast(
                        out=rrow[:], in_=pn_t[n2][:]
                    )
                    rrows.append(rrow)
                for n2 in range(ns):
                    srow = row_pool.tile([1, FD], CT, tag=f"sr{n2}")
                    nc.scalar.activation(srow[:], rrows[n2][:], AF.Sqrt)
                    srows.append(srow)

            def emit_repls():
                for n2 in range(ns):
                    pr = ps_repl.tile(
                        [P, FD], F32, tag="ps_repl", name=f"pr{n2}"
                    )
                    nc.tensor.matmul(pr[:], ones_row, srows[n2][:])
                    nc.scalar.activation(
                        rtn_bc[:, ds(n2 * FD, FD)], pr[:], AF.Copy
                    )

            pvs = proj_chains(
                kv, wvf, vif, chA,
                extra={4: emit_pn, 5: emit_rsqrt, 6: emit_repls},
            )
            proj_act([pvs[(0, m)] for m in range(mh)], bv_sb, vt_sb, 0)

            # --- fold 1/tn into tT (cheap bf16 DVE muls, run mid-phase)
            for n2 in range(ns):
                sl = ds(n2 * FD, FD)
                for m in range(mh):
                    nc.vector.tensor_mul(
                        tt_sb[:, m, sl], tt_sb[:, m, sl], rtn_bc[:, sl]
                    )

            # --- v squares half 0 + partial sums
            squares(vt_sb, vsq_sb, 0)
            chunk_sum(vsq_sb, vss_sb, 0)

            # --- proj-v phase B
            proj_chains(kv, wvf, vif, [(1, 2), (1, 3)], pvs=pvs,
                        pool=ps_repl, tag="ps_repl")
            proj_act([pvs[(1, m)] for m in range(mh)], bv_sb, vt_sb, 1)
            squares(vt_sb, vsq_sb, 1)
            chunk_sum(vsq_sb, vss_sb, 1)

            # --- dots + epilogue in two i-halves; each half preceded by its
            # 4 tiny vn2 column matmuls (stationary=vss block, moving=ones
            # column) -> approx 1/vn2 -> sqrt -> per-partition 1/vn columns
            def dots_block(i):
                out_sb = out_pool.tile([P, s], CT, name="out_sb", tag="out")
                pds = []
                for jc in range(ns):
                    pd = ps_mm.tile([P, FD], F32, tag="ps_mm", name="pd")
                    for hc in range(mh):
                        nc.tensor.matmul(
                            pd[:],
                            vt_sb[:, hc, ts(i, P)],
                            tt_sb[:, hc, ds(jc * FD, FD)],
                            start=(hc == 0),
                            stop=(hc == mh - 1),
                        )
                    pds.append(pd)
                return out_sb, pds

            def dots_epilogue(b, i, out_sb, pds):
                last = b == bpc - 1 and i == ms - 1
                for jc in range(ns):
                    pd = pds[jc]
                    if last:  # quarter the final epilogue for a short tail
                        h2 = FD // 2
                        for q in range(2):
                            osl = ds(jc * FD + q * h2, h2)
                            if q == 0:
                                nc.scalar.activation(
                                    out_sb[:, osl], pd[:, ds(q * h2, h2)],
                                    AF.Copy, scale=rvn_cols[:, ds(i, 1)],
                                )
                            else:
                                nc.vector.tensor_scalar_mul(
                                    out_sb[:, osl], pd[:, ds(q * h2, h2)],
                                    rvn_cols[:, ds(i, 1)],
                                )
                    elif jc == 0:  # split epilogues across Scalar and DVE
                        nc.scalar.activation(
                            out_sb[:, ds(jc * FD, FD)], pd[:], AF.Copy,
                            scale=rvn_cols[:, ds(i, 1)],
                        )
                    else:
                        nc.vector.tensor_scalar_mul(
                            out_sb[:, ds(jc * FD, FD)], pd[:],
                            rvn_cols[:, ds(i, 1)],
                        )
                if last:
                    lastq = [nc.sync, nc.gpsimd, nc.scalar, nc.sync]
                    for q4 in range(4):
                        lastq[q4].dma_start(
                            out[b, ds(i * P, P), ds(q4 * FD // 2, FD // 2)],
                            out_sb[:, ds(q4 * FD // 2, FD // 2)],
                        )
                else:
                    oq = nc.sync if (b == bpc - 1 and i >= ms // 2) else nc.gpsimd
                    oq.dma_start(out[b, ds(i * P, P), :], out_sb[:])

            for half in range(2):
                csl = ds(half * (ms // 2), ms // 2)
                i0 = half * (ms // 2)
                # first dots block of the half runs while the vn2 column
                # chain (which needs the DVE partial sums) catches up
                o0, pds0 = dots_block(i0)
                pcol = ps_mm.tile([P, ms // 2], F32, tag="ps_mm", name=f"pc{half}")
                for sb in range(ms // 2):
                    i = i0 + sb
                    nc.tensor.matmul(
                        pcol[:, ds(sb, 1)], vss_sb[:, ts(i, P)], ones_col
                    )
                ctmp = col_pool.tile([P, ms // 2], F32, tag=f"ctmp{half}")
                nc.vector.reciprocal_approx_fast(out=ctmp[:], in_=pcol[:])
                nc.scalar.activation(rvn_cols[:, csl], ctmp[:], AF.Sqrt)
                dots_epilogue(b, i0, o0, pds0)

                for sb in range(1, ms // 2):
                    i = i0 + sb
                    out_sb, pds = dots_block(i)
                    dots_epilogue(b, i, out_sb, pds)

    nc.compile()
    return nc


_CACHE = {}


def _get_nc():
    if "nc" not in _CACHE:
        _CACHE["nc"] = build()
    return _CACHE["nc"]


def _prep_in_maps(visual_features, text_features, Wv, bv, Wt, bt):
    import ml_dtypes

    f = np.float32
    ct = ml_dtypes.bfloat16
    kv, kt = VD // P, TD // P
    wvT = np.ascontiguousarray(np.asarray(Wv, dtype=f).T).astype(ct)  # [VD, H]
    wtT = np.ascontiguousarray(np.asarray(Wt, dtype=f).T).astype(ct)  # [TD, H]
    # chunked weight layouts: [P, k*H] with chunk k at cols [k*H,(k+1)*H)
    wt_c = wtT.reshape(kt, P, H)                      # [kt, P, H]
    wt0 = np.ascontiguousarray(wt_c[0])               # [P, H]
    wtr = np.ascontiguousarray(
        wt_c[1:].transpose(1, 0, 2).reshape(P, (kt - 1) * H))
    wv_c = wvT.reshape(kv, P, H)
    wva = np.ascontiguousarray(
        wv_c[: kv // 2].transpose(1, 0, 2).reshape(P, (kv // 2) * H))
    wvb = np.ascontiguousarray(
        wv_c[kv // 2:].transpose(1, 0, 2).reshape(P, (kv // 2) * H))
    bvp = np.ascontiguousarray(np.asarray(bv, dtype=f).reshape(H // P, P).T)
    btp = np.ascontiguousarray(np.asarray(bt, dtype=f).reshape(H // P, P).T)
    ones = np.ones((P, P), dtype=f).astype(ct)
    vis = np.asarray(visual_features, dtype=f)
    txt = np.asarray(text_features, dtype=f)
    in_maps = []
    for c in range(NCORES):
        b0, b1 = c * BPC, c * BPC + 1
        # [S, D] -> [D, S] -> chunked [k, P, S]
        t0 = txt[b0].T.astype(ct).reshape(kt, P, S)
        t1 = txt[b1].T.astype(ct).reshape(kt, P, S)
        v0 = vis[b0].T.astype(ct).reshape(kv, P, S)
        v1 = vis[b1].T.astype(ct).reshape(kv, P, S)
        vq = v0.reshape(4, kv // 4, P, S).transpose(0, 2, 1, 3)
        in_maps.append({
            "tx00": np.ascontiguousarray(t0[0]),
            "tx0r": np.ascontiguousarray(t0[1:]),
            "tx1": np.ascontiguousarray(
                t1.transpose(1, 0, 2).reshape(P, kt * S)),
            "vi0q": np.ascontiguousarray(
                vq.reshape(4, P, (kv // 4) * S)),
            "vi1": np.ascontiguousarray(
                v1.transpose(1, 0, 2).reshape(P, kv * S)),
            "wt0": wt0,
            "wtr": wtr,
            "wva": wva,
            "wvb": wvb,
            "bvp": bvp,
            "btp": btp,
            "ones": ones,
        })
    return in_maps


def run(inputs, trace=False, tmpdir=None):
    """Returns (full_output, BassKernelResults)."""
    from concourse.bass_utils import run_bass_kernel_spmd

    nc = _get_nc()
    in_maps = _prep_in_maps(**inputs)
    res = run_bass_kernel_spmd(
        nc, in_maps, core_ids=list(range(NCORES)), trace=trace, tmpdir=tmpdir
    )
    outp = np.concatenate(
        [np.asarray(res.results[c]["out"]) for c in range(NCORES)], axis=0
    ).astype(np.float32)
    return outp, res


def kernel(**inputs) -> np.ndarray:
    outp, _ = run(inputs, trace=False)
    return outp
